# revision 12
# baseline (speedup 1.0000x reference)
"""Trainium2 Bass kernel for nn_ContConv1dDenseSim (banded continuous conv).

Math (reference):
  dt[b,l,j] = times[b,l]-times[b,j], masked to a causal band j in [l-W+1, l]
  (W = (sim_size+1)*kernel_size = 30), true_ids[b,j], and a row-validity mask.
  h = relu(dt*w1+b1)  (8 hidden), kv = (h@w2+b2) masked, reshaped (16,16)
  out[b,l,o] = sum_{j,i} features[b,j,i] * kv[b,l,j,i,o]

Factorization:
  G[j,k*16+o] = sum_i (f[j,i]*tiw[j]) * W2p[i,k*16+o]   (k=0..7; col 128+o is
                the b2 bias channel)
  A_k[jl,p]   = band(jl,p) * relu(w1k*dt[p,jl] + b1k)
  out[p,o]    = rv[p] * sum_{k,jl} A_k[jl,p] * G[jl,k*16+o]

Sharding: 8 cores = 2 batches x 4 query-row blocks of 128. Each core sees a
157-column key window (128 "up" + 29 "lo" fold) and emits out^T (16,128);
the host transposes back.

Device program (per core):
  - ONE packed input DMA (16 x 1712 f32).
  - band masks from 3 gpsimd affine_selects ([128,128] up, [29,128] lo),
    generated during the input-DMA latency window.
  - G via 2 PE matmuls (featT*tiw stationary, W2p moving), PSUM->SBUF copies
    on the scalar engine.
  - the whole pre-activation PA[jl,(k,p)] = w1k*dt+b1k via a single K=2
    matmul pair per fold (lhsT = [ones; t_win], moving = host-packed
    [w1k*t_row+b1k ; -w1k] rows), N=512 x 2.
  - fused mask+relu: A = max(PA,0)*band in one scalar_tensor_tensor per
    512-wide half (vector engine for the up fold, gpsimd for lo), with the
    [*,128] band broadcast across the 4 k-blocks by a stride-0 AP.
  - contraction with G STATIONARY (LDWEIGHTS P=16) and A MOVING (N=128):
    18 PSUM-accumulated matmuls into out^T[16,128].
  - rv row-validity fold via one vector multiply, DMA out.
"""

import numpy as np
import concourse.bass as bass
import concourse.tile as tile
import concourse.mybir as mybir
from concourse.bass_utils import run_bass_kernel_spmd
from concourse.tile_rust import add_dep_helper

F32 = mybir.dt.float32
Alu = mybir.AluOpType

BS, L, CH, HID, KS = 2, 512, 16, 8, 5
LBLK = 128                      # query rows per core
NBLK = L // LBLK                # 4
NCORES = BS * NBLK              # 8
NF = (HID + 1) * CH             # 144 G columns (8 hidden + b2 channel)
PAW = HID * LBLK                # 1024 wide pre-activation columns

# packed input column layout
C_FW = 0                        # [16, 256]  featT*tiw, folded (up|lo+pad)
C_W2 = 256                      # [16, 144]  W2p
C_RHS = 400                     # [2, 1024]  PA moving rows
C_LUP = 1424                    # [2, 128]   PA lhsT up (ones; t_win)
C_LLO = 1552                    # [2, 29]    PA lhsT lo
C_RV = 1584                     # [1, 128]   row-validity
NCOL = 1712

TRACE = False
LAST = None

_prog_cache = {}


def _build(W):
    WIN = LBLK + W - 1          # 157
    LO = WIN - 128              # 29
    nc = bass.Bass(trn_type="TRN2")

    big = nc.declare_dram_parameter("big", [16, NCOL], F32, isOutput=False)
    out_d = nc.declare_dram_parameter("out", [CH, LBLK], F32, isOutput=True)

    with tile.TileContext(nc) as tc:
        with (
            tc.tile_pool(name="sb", bufs=1) as sb,
            tc.tile_pool(name="ps", bufs=1, space="PSUM") as ps,
        ):
            t_big = sb.tile([16, NCOL], F32)
            dma_in = nc.sync.dma_start(t_big[:], big[:])
            fwT = t_big[:, C_FW:C_FW + 256]
            w2p = t_big[:, C_W2:C_W2 + NF]
            rhsw = t_big[0:2, C_RHS:C_RHS + PAW]
            lhs_up = t_big[0:2, C_LUP:C_LUP + 128]
            lhs_lo = t_big[0:2, C_LLO:C_LLO + LO]
            rv = t_big[:, C_RV:C_RV + 128]

            # ---- band masks (input-independent; hidden under DMA wait) ----
            ones = sb.tile([128, 128], F32)
            nc.gpsimd.memset(ones[:], 1.0)
            btmp = sb.tile([128, 128], F32)
            band_up = sb.tile([128, 128], F32)
            band_lo = sb.tile([LO, 128], F32)
            # up fold: partition=jl, free=p; keep iff 0 <= jl - p <= W-1
            nc.gpsimd.affine_select(btmp[:], ones[:], [[-1, 128]],
                                    Alu.is_ge, 0.0, base=0,
                                    channel_multiplier=1)
            nc.gpsimd.affine_select(band_up[:], btmp[:], [[1, 128]],
                                    Alu.is_ge, 0.0, base=W - 1,
                                    channel_multiplier=-1)
            # lo fold: jl = 128 + q; jl - p >= 0 is vacuous, keep iff
            # (W-1) - (128+q) + p >= 0
            last_gp = nc.gpsimd.affine_select(band_lo[:], ones[0:LO, :],
                                              [[1, 128]], Alu.is_ge, 0.0,
                                              base=W - 1 - 128,
                                              channel_multiplier=-1)

            # ---- G[jl, k*16+o], both folds, then PSUM->SBUF on scalar ----
            p_g = ps.tile([128, 2 * NF], F32)
            nc.tensor.matmul(p_g[:, 0:NF], fwT[:, 0:128], w2p[:],
                             start=True, stop=True)
            nc.tensor.matmul(p_g[0:LO, NF:2 * NF], fwT[:, 128:128 + LO],
                             w2p[:], start=True, stop=True)
            g_up = sb.tile([128, NF], F32)
            g_lo = sb.tile([LO, NF], F32)
            nc.scalar.copy(g_up[:], p_g[:, 0:NF])
            last_sc = nc.scalar.copy(g_lo[:], p_g[0:LO, NF:2 * NF])

            # ---- PA[jl, (k,p)] = w1k*dt + b1k, one K=2 matmul per half ----
            p_au = ps.tile([128, PAW], F32)
            p_al = ps.tile([LO, PAW], F32)
            nc.tensor.matmul(p_au[:, 0:512], lhs_up[:], rhsw[:, 0:512],
                             start=True, stop=True)
            nc.tensor.matmul(p_au[:, 512:1024], lhs_up[:], rhsw[:, 512:1024],
                             start=True, stop=True)
            nc.tensor.matmul(p_al[:, 0:512], lhs_lo[:], rhsw[:, 0:512],
                             start=True, stop=True)
            nc.tensor.matmul(p_al[:, 512:1024], lhs_lo[:], rhsw[:, 512:1024],
                             start=True, stop=True)

            # ---- observers: TRN2 instructions encode ONE sync wait, so
            # each engine's first touch of a foreign tensor must be an op
            # with a single new cross-engine dep. ----
            # PE absorbs the gpsimd band sem (covers both bands).
            p_obs = ps.tile([1, 8], F32)
            nc.tensor.matmul(p_obs[0:1, 0:1], band_up[0:1, 0:1],
                             band_up[0:1, 0:1], start=True, stop=True)
            # Vector absorbs the DMA sem then the gpsimd sem.
            obs_v = sb.tile([1, 2], F32)
            nc.vector.tensor_copy(obs_v[:, 0:1], t_big[0:1, 0:1])
            nc.vector.tensor_copy(obs_v[:, 1:2], band_up[0:1, 0:1])

            # ---- A = max(PA,0) * band, fused, on vector ----
            a_up = sb.tile([128, PAW], F32)
            a_lo = sb.tile([LO, PAW], F32)
            bu3 = band_up[:].unsqueeze(1).broadcast_to((128, 4, 128))
            bl3 = band_lo[:].unsqueeze(1).broadcast_to((LO, 4, 128))
            for h in range(2):
                s = slice(h * 512, (h + 1) * 512)
                nc.vector.scalar_tensor_tensor(
                    a_up[:, s].rearrange("a (k p) -> a k p", p=128),
                    p_au[:, s].rearrange("a (k p) -> a k p", p=128),
                    0.0, bu3, Alu.max, Alu.mult)
                nc.vector.scalar_tensor_tensor(
                    a_lo[:, s].rearrange("a (k p) -> a k p", p=128),
                    p_al[:, s].rearrange("a (k p) -> a k p", p=128),
                    0.0, bl3, Alu.max, Alu.mult)

            # ---- out^T[o,p] = sum_{k,jl} G[jl,ko] * A_k[jl,p] ----
            # G slices are STATIONARY (LDWEIGHTS P=16), A moving (N=128).
            p_out = ps.tile([CH, 128], F32)
            nc.tensor.matmul(p_out[:], g_up[:, HID * CH:NF], band_up[:],
                             start=True, stop=False)
            nc.tensor.matmul(p_out[:], g_lo[:, HID * CH:NF], band_lo[:],
                             start=False, stop=False)
            last_pe = None
            for k in range(HID):
                nc.tensor.matmul(p_out[:], g_up[:, k * CH:(k + 1) * CH],
                                 a_up[:, k * 128:(k + 1) * 128],
                                 start=False, stop=False)
                last_pe = nc.tensor.matmul(
                    p_out[:], g_lo[:, k * CH:(k + 1) * CH],
                    a_lo[:, k * 128:(k + 1) * 128],
                    start=False, stop=(k == HID - 1))

            # ---- row-validity fold + store (host transposes back) ----
            o_sb = sb.tile([CH, 128], F32)
            last_ve = nc.vector.tensor_mul(o_sb[:], p_out[:], rv)
            dma_out = nc.sync.dma_start(out_d[:], o_sb[:])

            # The kernel-tail drain waits on every sem; TRN2 instructions
            # encode one wait each, so pre-satisfy with single-wait SP nops.
            for prod in (dma_in, dma_out, last_gp, last_sc, last_ve,
                         last_pe):
                nop = nc.sync.nop(nofuse=True, hint="predrain_observer")
                add_dep_helper(nop.ins, prod.ins, sync=True,
                               reason="pre-drain single-wait observer")
    return nc


def kernel(times, features, lengths, true_ids, sim_size, w1, b1, w2, b2):
    global LAST
    times = np.ascontiguousarray(np.asarray(times, dtype=np.float32))
    features = np.ascontiguousarray(np.asarray(features, dtype=np.float32))
    lengths = np.asarray(lengths)
    true_ids = np.asarray(true_ids)
    sim = int(np.asarray(sim_size))
    w1 = np.asarray(w1, dtype=np.float32).reshape(-1)
    b1 = np.asarray(b1, dtype=np.float32).reshape(-1)
    w2 = np.asarray(w2, dtype=np.float32)
    b2 = np.asarray(b2, dtype=np.float32)

    W = (sim + 1) * KS
    WIN = LBLK + W - 1
    LO = WIN - 128

    if W not in _prog_cache:
        _prog_cache[W] = _build(W)
    nc = _prog_cache[W]

    # W2p[i, k*16+o] = w2[k, i*16+o]; cols 128:144 = b2 reshaped (16,16)
    w2p = np.concatenate(
        [w2.reshape(HID, CH, CH).transpose(1, 0, 2).reshape(CH, HID * CH),
         b2.reshape(CH, CH)], axis=1).astype(np.float32)

    in_maps = []
    for core in range(NCORES):
        b, blk = divmod(core, NBLK)
        l0 = blk * LBLK
        idx = np.arange(l0 - (W - 1), l0 + LBLK)
        valid = idx >= 0
        idxc = np.clip(idx, 0, L - 1)
        t_win = np.where(valid, times[b, idxc], 0.0).astype(np.float32)
        tiw = (true_ids[b, idxc] & valid).astype(np.float32)
        feat_w = (np.where(valid[:, None], features[b, idxc, :], 0.0)
                  * tiw[:, None]).astype(np.float32)
        t_row = times[b, l0:l0 + LBLK].astype(np.float32)
        rv = (np.arange(l0, l0 + LBLK) <=
              (sim + 1) * (int(lengths[b]) - 1)).astype(np.float32)

        big = np.zeros((16, NCOL), np.float32)
        big[:, C_FW:C_FW + 128] = feat_w[0:128].T
        big[:, C_FW + 128:C_FW + 128 + LO] = feat_w[128:WIN].T
        big[:, C_W2:C_W2 + NF] = w2p
        big[0, C_RHS:C_RHS + PAW] = (w1[:, None] * t_row[None, :]
                                     + b1[:, None]).reshape(-1)
        big[1, C_RHS:C_RHS + PAW] = np.repeat(-w1, 128)
        big[0, C_LUP:C_LUP + 128] = 1.0
        big[1, C_LUP:C_LUP + 128] = t_win[0:128]
        big[0, C_LLO:C_LLO + LO] = 1.0
        big[1, C_LLO:C_LLO + LO] = t_win[128:WIN]
        big[:, C_RV:C_RV + 128] = rv[None, :]
        in_maps.append({"big": big})

    res = run_bass_kernel_spmd(nc, in_maps, core_ids=list(range(NCORES)),
                               trace=TRACE)
    LAST = res

    out = np.zeros((BS, L, CH), np.float32)
    for core in range(NCORES):
        b, blk = divmod(core, NBLK)
        out[b, blk * LBLK:(blk + 1) * LBLK, :] = res.results[core]["out"].T
    return out


# revision 13
# speedup vs baseline: 1.5603x; 1.5603x over previous
"""Trainium2 Bass kernel for nn_ContConv1dDenseSim (banded continuous conv).

Math (reference):
  dt[b,l,j] = times[b,l]-times[b,j], masked to a causal band j in [l-W+1, l]
  (W = (sim_size+1)*kernel_size = 30), true_ids[b,j], and a row-validity mask.
  h = relu(dt*w1+b1)  (8 hidden), kv = (h@w2+b2) masked, reshaped (16,16)
  out[b,l,o] = sum_{j,i} features[b,j,i] * kv[b,l,j,i,o]

Factorization:
  G[j,k*16+o] = sum_i (f[j,i]*tiw[j]) * W2p[i,k*16+o]   (k=0..7; col 128+o is
                the b2 bias channel)
  A_k[jl,p]   = band(jl,p) * relu(w1k*dt[p,jl] + b1k)
  out[p,o]    = rv[p] * sum_{k,jl} A_k[jl,p] * G[jl,k*16+o]

Sharding: 8 cores = 2 batches x 4 query-row blocks of 128. Each core sees a
157-column key window (128 "up" + 29 "lo" fold) and emits out^T (16,128);
the host transposes back.

Device program (per core), all moving matmul operands in BF16 (1 cyc/col on
the PE vs 2 for fp32):
  - ONE packed input DMA (16 x 1120 f32; bf16 sections bit-packed on host and
    bitcast on device).
  - band masks from 3 gpsimd affine_selects (bf16 [128,128] up, [29,128] lo),
    generated during the input-DMA latency window.
  - G via 2 PE matmuls (featT*tiw stationary fp32, W2p moving), PSUM->SBUF
    bf16 copies on the scalar engine.
  - the whole pre-activation PA[jl,(k,p)] = w1k*dt+b1k via K=2 bf16 matmuls
    (lhsT = [ones; t_win], moving = host-packed [w1k*t_row+b1k ; -w1k] rows).
  - fused mask+relu on vector: A = max(PA,0)*band per 512-wide half, bf16
    out, with the [*,128] band broadcast across k-blocks by a stride-0 AP.
  - contraction with G STATIONARY (LDWEIGHTS P=16) and A MOVING (N=128 bf16):
    18 PSUM-accumulated matmuls into out^T[16,128].
  - rv row-validity fold via one vector multiply, DMA out.

NOTE: TRN2 engine instructions encode a single sync-wait, so the program is
ordered so each engine's first touch of any foreign-produced tensor has
exactly one new cross-engine dependency (tiny observer ops where needed), and
the Tile kernel-tail drain is pre-satisfied by single-wait SP nops.
"""

import numpy as np
import ml_dtypes
import concourse.bass as bass
import concourse.tile as tile
import concourse.mybir as mybir
from concourse.bass_utils import run_bass_kernel_spmd
from concourse.tile_rust import add_dep_helper

F32 = mybir.dt.float32
BF16 = mybir.dt.bfloat16
Alu = mybir.AluOpType

BS, L, CH, HID, KS = 2, 512, 16, 8, 5
LBLK = 128                      # query rows per core
NBLK = L // LBLK                # 4
NCORES = BS * NBLK              # 8
NF = (HID + 1) * CH             # 144 G columns (8 hidden + b2 channel)
PAW = HID * LBLK                # 1024 wide pre-activation columns

# packed input column layout (f32 columns; *B sections hold 2 bf16 per col)
C_FW = 0                        # [16, 256]  featT*tiw, folded (up|lo+pad)
C_W2 = 256                      # [16, 144]  W2p
C_RHSB = 400                    # [2, 512]   PA moving rows, bf16-packed
C_LUPB = 912                    # [2, 64]    PA lhsT up (ones; t_win), bf16
C_LLOB = 976                    # [2, 16]    PA lhsT lo, bf16-packed
C_RV = 992                      # [16, 128]  row-validity (replicated)
NCOL = 1120

TRACE = False
LAST = None

_prog_cache = {}


def _pack_bf16(x):
    """Pack a float array into f32 'containers', 2 bf16 per f32 column."""
    bf = np.ascontiguousarray(x.astype(ml_dtypes.bfloat16))
    v = bf.view(np.uint16)
    u32 = (v[..., 0::2].astype(np.uint32)
           | (v[..., 1::2].astype(np.uint32) << 16))
    return u32.view(np.float32)


def _build(W):
    WIN = LBLK + W - 1          # 157
    LO = WIN - 128              # 29
    nc = bass.Bass(trn_type="TRN2")

    big = nc.declare_dram_parameter("big", [16, NCOL], F32, isOutput=False)
    out_d = nc.declare_dram_parameter("out", [CH, LBLK], F32, isOutput=True)

    with tile.TileContext(nc) as tc:
        with (
            tc.tile_pool(name="sb", bufs=1) as sb,
            tc.tile_pool(name="ps", bufs=1, space="PSUM") as ps,
        ):
            t_big = sb.tile([16, NCOL], F32)
            dma_in = nc.sync.dma_start(t_big[:], big[:])
            fwT = t_big[:, C_FW:C_FW + 256]
            w2p = t_big[:, C_W2:C_W2 + NF]
            rhsw = t_big[0:2, C_RHSB:C_RHSB + 512].bitcast(BF16)  # [2,1024]
            lhs_up = t_big[0:2, C_LUPB:C_LUPB + 64].bitcast(BF16)  # [2,128]
            lhs_lo = t_big[0:2, C_LLOB:C_LLOB + 16].bitcast(BF16)  # [2,32]
            rv = t_big[:, C_RV:C_RV + 128]

            # ---- band masks (input-independent; hidden under DMA wait) ----
            ones = sb.tile([128, 128], BF16)
            nc.gpsimd.memset(ones[:], 1.0)
            btmp = sb.tile([128, 128], BF16)
            band_up = sb.tile([128, 128], BF16)
            band_lo = sb.tile([LO, 128], BF16)
            # up fold: partition=jl, free=p; keep iff 0 <= jl - p <= W-1
            nc.gpsimd.affine_select(btmp[:], ones[:], [[-1, 128]],
                                    Alu.is_ge, 0.0, base=0,
                                    channel_multiplier=1)
            nc.gpsimd.affine_select(band_up[:], btmp[:], [[1, 128]],
                                    Alu.is_ge, 0.0, base=W - 1,
                                    channel_multiplier=-1)
            # lo fold: jl = 128 + q; jl - p >= 0 is vacuous, keep iff
            # (W-1) - (128+q) + p >= 0
            last_gp = nc.gpsimd.affine_select(band_lo[:], ones[0:LO, :],
                                              [[1, 128]], Alu.is_ge, 0.0,
                                              base=W - 1 - 128,
                                              channel_multiplier=-1)

            # ---- G[jl, k*16+o], both folds, PSUM -> bf16 SBUF on scalar ----
            p_g = ps.tile([128, 2 * NF], F32)
            nc.tensor.matmul(p_g[:, 0:NF], fwT[:, 0:128], w2p[:],
                             start=True, stop=True)
            nc.tensor.matmul(p_g[0:LO, NF:2 * NF], fwT[:, 128:128 + LO],
                             w2p[:], start=True, stop=True)
            g_up = sb.tile([128, NF], BF16)
            g_lo = sb.tile([LO, NF], BF16)
            nc.scalar.copy(g_up[:], p_g[:, 0:NF])
            last_sc = nc.scalar.copy(g_lo[:], p_g[0:LO, NF:2 * NF])

            # ---- observers (single-wait discipline, see module docstring) --
            # PE absorbs the gpsimd band sem (covers both bands).
            p_obs = ps.tile([1, 8], F32)
            nc.tensor.matmul(p_obs[0:1, 0:1], band_up[0:1, 0:1],
                             band_up[0:1, 0:1], start=True, stop=True)
            # Vector absorbs the DMA sem then the gpsimd sem.
            obs_v = sb.tile([1, 2], F32)
            nc.vector.tensor_copy(obs_v[:, 0:1], t_big[0:1, 0:1])
            nc.vector.tensor_copy(obs_v[:, 1:2], band_up[0:1, 0:1])

            # ---- PA[jl, (k,p)] = w1k*dt + b1k, K=2 bf16 matmuls ----
            p_au = ps.tile([128, PAW], F32)
            p_al = ps.tile([LO, PAW], F32)
            nc.tensor.matmul(p_au[:, 0:512], lhs_up[:], rhsw[:, 0:512],
                             start=True, stop=True)
            nc.tensor.matmul(p_au[:, 512:1024], lhs_up[:], rhsw[:, 512:1024],
                             start=True, stop=True)
            nc.tensor.matmul(p_al[:, 0:512], lhs_lo[:, 0:LO], rhsw[:, 0:512],
                             start=True, stop=True)
            nc.tensor.matmul(p_al[:, 512:1024], lhs_lo[:, 0:LO],
                             rhsw[:, 512:1024], start=True, stop=True)

            # ---- A = max(PA,0) * band, fused, bf16 out, on vector ----
            a_up = sb.tile([128, PAW], BF16)
            a_lo = sb.tile([LO, PAW], BF16)
            bu3 = band_up[:].unsqueeze(1).broadcast_to((128, 4, 128))
            bl3 = band_lo[:].unsqueeze(1).broadcast_to((LO, 4, 128))
            stt = []
            for h in range(2):
                s = slice(h * 512, (h + 1) * 512)
                stt.append(nc.vector.scalar_tensor_tensor(
                    a_up[:, s].rearrange("a (k p) -> a k p", p=128),
                    p_au[:, s].rearrange("a (k p) -> a k p", p=128),
                    0.0, bu3, Alu.max, Alu.mult))
            for h in range(2):
                s = slice(h * 512, (h + 1) * 512)
                stt.append(nc.vector.scalar_tensor_tensor(
                    a_lo[:, s].rearrange("a (k p) -> a k p", p=128),
                    p_al[:, s].rearrange("a (k p) -> a k p", p=128),
                    0.0, bl3, Alu.max, Alu.mult))

            # ---- out^T[o,p] = sum_{k,jl} G[jl,ko] * A_k[jl,p] ----
            # G slices STATIONARY (LDWEIGHTS P=16), A moving (N=128 bf16).
            p_out = ps.tile([CH, 128], F32)
            nc.tensor.matmul(p_out[:], g_up[:, HID * CH:NF], band_up[:],
                             start=True, stop=False)
            nc.tensor.matmul(p_out[:], g_lo[:, HID * CH:NF], band_lo[:],
                             start=False, stop=False)
            for k in range(HID):          # up fold, gated on stt[0]/stt[1]
                nc.tensor.matmul(p_out[:], g_up[:, k * CH:(k + 1) * CH],
                                 a_up[:, k * 128:(k + 1) * 128],
                                 start=False, stop=False)
            last_pe = None
            for k in range(HID):          # lo fold, gated on stt[2]/stt[3]
                last_pe = nc.tensor.matmul(
                    p_out[:], g_lo[:, k * CH:(k + 1) * CH],
                    a_lo[:, k * 128:(k + 1) * 128],
                    start=False, stop=(k == HID - 1))

            # ---- row-validity fold + store (host transposes back) ----
            o_sb = sb.tile([CH, 128], F32)
            last_ve = nc.vector.tensor_mul(o_sb[:], p_out[:], rv)
            dma_out = nc.sync.dma_start(out_d[:], o_sb[:])

            # The kernel-tail drain waits on every sem; TRN2 instructions
            # encode one wait each, so pre-satisfy with single-wait SP nops.
            for prod in (dma_in, dma_out, last_gp, last_sc, last_ve,
                         last_pe):
                nop = nc.sync.nop(nofuse=True, hint="predrain_observer")
                add_dep_helper(nop.ins, prod.ins, sync=True,
                               reason="pre-drain single-wait observer")
    return nc


def kernel(times, features, lengths, true_ids, sim_size, w1, b1, w2, b2):
    global LAST
    times = np.ascontiguousarray(np.asarray(times, dtype=np.float32))
    features = np.ascontiguousarray(np.asarray(features, dtype=np.float32))
    lengths = np.asarray(lengths)
    true_ids = np.asarray(true_ids)
    sim = int(np.asarray(sim_size))
    w1 = np.asarray(w1, dtype=np.float32).reshape(-1)
    b1 = np.asarray(b1, dtype=np.float32).reshape(-1)
    w2 = np.asarray(w2, dtype=np.float32)
    b2 = np.asarray(b2, dtype=np.float32)

    W = (sim + 1) * KS
    WIN = LBLK + W - 1
    LO = WIN - 128

    if W not in _prog_cache:
        _prog_cache[W] = _build(W)
    nc = _prog_cache[W]

    # W2p[i, k*16+o] = w2[k, i*16+o]; cols 128:144 = b2 reshaped (16,16)
    w2p = np.concatenate(
        [w2.reshape(HID, CH, CH).transpose(1, 0, 2).reshape(CH, HID * CH),
         b2.reshape(CH, CH)], axis=1).astype(np.float32)

    in_maps = []
    for core in range(NCORES):
        b, blk = divmod(core, NBLK)
        l0 = blk * LBLK
        idx = np.arange(l0 - (W - 1), l0 + LBLK)
        valid = idx >= 0
        idxc = np.clip(idx, 0, L - 1)
        t_win = np.where(valid, times[b, idxc], 0.0).astype(np.float32)
        tiw = (true_ids[b, idxc] & valid).astype(np.float32)
        feat_w = (np.where(valid[:, None], features[b, idxc, :], 0.0)
                  * tiw[:, None]).astype(np.float32)
        t_row = times[b, l0:l0 + LBLK].astype(np.float32)
        rv = (np.arange(l0, l0 + LBLK) <=
              (sim + 1) * (int(lengths[b]) - 1)).astype(np.float32)

        big = np.zeros((16, NCOL), np.float32)
        big[:, C_FW:C_FW + 128] = feat_w[0:128].T
        big[:, C_FW + 128:C_FW + 128 + LO] = feat_w[128:WIN].T
        big[:, C_W2:C_W2 + NF] = w2p
        pa_rows = np.stack([
            (w1[:, None] * t_row[None, :] + b1[:, None]).reshape(-1),
            np.repeat(-w1, 128)])
        big[0:2, C_RHSB:C_RHSB + 512] = _pack_bf16(pa_rows)
        lu = np.stack([np.ones(128, np.float32), t_win[0:128]])
        big[0:2, C_LUPB:C_LUPB + 64] = _pack_bf16(lu)
        ll = np.zeros((2, 32), np.float32)
        ll[0, 0:LO] = 1.0
        ll[1, 0:LO] = t_win[128:WIN]
        big[0:2, C_LLOB:C_LLOB + 16] = _pack_bf16(ll)
        big[:, C_RV:C_RV + 128] = rv[None, :]
        in_maps.append({"big": big})

    res = run_bass_kernel_spmd(nc, in_maps, core_ids=list(range(NCORES)),
                               trace=TRACE)
    LAST = res

    out = np.zeros((BS, L, CH), np.float32)
    for core in range(NCORES):
        b, blk = divmod(core, NBLK)
        out[b, blk * LBLK:(blk + 1) * LBLK, :] = res.results[core]["out"].T
    return out


# revision 14
# speedup vs baseline: 1.5950x; 1.0222x over previous
"""Trainium2 Bass kernel for nn_ContConv1dDenseSim (banded continuous conv).

Math (reference):
  dt[b,l,j] = times[b,l]-times[b,j], masked to a causal band j in [l-W+1, l]
  (W = (sim_size+1)*kernel_size = 30), true_ids[b,j], and a row-validity mask.
  h = relu(dt*w1+b1)  (8 hidden), kv = (h@w2+b2) masked, reshaped (16,16)
  out[b,l,o] = sum_{j,i} features[b,j,i] * kv[b,l,j,i,o]

Factorization:
  G[j,k*16+o] = sum_i (f[j,i]*tiw[j]) * W2p[i,k*16+o]   (k=0..7; col 128+o is
                the b2 bias channel)
  A_k[jl,p]   = band(jl,p) * relu(w1k*dt[p,jl] + b1k)
  out[p,o]    = rv[p] * sum_{k,jl} A_k[jl,p] * G[jl,k*16+o]

Sharding: 8 cores = 2 batches x 4 query-row blocks of 128. Each core sees a
157-column key window (128 "up" + 29 "lo" fold) and emits out^T (16,128);
the host transposes back.

Device program (per core), all moving matmul operands in BF16 (1 cyc/col on
the PE vs 2 for fp32):
  - ONE packed input DMA (16 x 1120 f32; bf16 sections bit-packed on host and
    bitcast on device).
  - band masks from 3 gpsimd affine_selects (bf16 [128,128] up, [29,128] lo),
    generated during the input-DMA latency window.
  - G via 2 PE matmuls (featT*tiw stationary fp32, W2p moving), PSUM->SBUF
    bf16 copies on the scalar engine.
  - the whole pre-activation PA[jl,(k,p)] = w1k*dt+b1k via K=2 bf16 matmuls
    (lhsT = [ones; t_win], moving = host-packed [w1k*t_row+b1k ; -w1k] rows).
  - fused mask+relu on vector: A = max(PA,0)*band per 512-wide half, bf16
    out, with the [*,128] band broadcast across k-blocks by a stride-0 AP.
  - contraction with G STATIONARY (LDWEIGHTS P=16) and A MOVING (N=128 bf16):
    18 PSUM-accumulated matmuls into out^T[16,128].
  - rv row-validity fold via one vector multiply, DMA out.

NOTE: TRN2 engine instructions encode a single sync-wait, so the program is
ordered so each engine's first touch of any foreign-produced tensor has
exactly one new cross-engine dependency (tiny observer ops where needed), and
the Tile kernel-tail drain is pre-satisfied by single-wait SP nops.
"""

import numpy as np
import ml_dtypes
import concourse.bass as bass
import concourse.tile as tile
import concourse.mybir as mybir
from concourse.bass_utils import run_bass_kernel_spmd
from concourse.tile_rust import add_dep_helper

F32 = mybir.dt.float32
BF16 = mybir.dt.bfloat16
Alu = mybir.AluOpType

BS, L, CH, HID, KS = 2, 512, 16, 8, 5
LBLK = 128                      # query rows per core
NBLK = L // LBLK                # 4
NCORES = BS * NBLK              # 8
NF = (HID + 1) * CH             # 144 G columns (8 hidden + b2 channel)
PAW = HID * LBLK                # 1024 wide pre-activation columns

# packed input column layout (f32 columns; *B sections hold 2 bf16 per col)
C_FW = 0                        # [16, 256]  featT*tiw, folded (up|lo+pad)
C_W2 = 256                      # [16, 144]  W2p
C_RHSB = 400                    # [2, 512]   PA moving rows, bf16-packed
C_LUPB = 912                    # [2, 64]    PA lhsT up (ones; t_win), bf16
C_LLOB = 976                    # [2, 16]    PA lhsT lo, bf16-packed
C_RV = 992                      # [16, 128]  row-validity (replicated)
NCOL = 1120

TRACE = False
LAST = None

_prog_cache = {}


def _pack_bf16(x):
    """Pack a float array into f32 'containers', 2 bf16 per f32 column."""
    bf = np.ascontiguousarray(x.astype(ml_dtypes.bfloat16))
    v = bf.view(np.uint16)
    u32 = (v[..., 0::2].astype(np.uint32)
           | (v[..., 1::2].astype(np.uint32) << 16))
    return u32.view(np.float32)


def _build(W):
    WIN = LBLK + W - 1          # 157
    LO = WIN - 128              # 29
    nc = bass.Bass(trn_type="TRN2")

    big = nc.declare_dram_parameter("big", [16, NCOL], F32, isOutput=False)
    out_d = nc.declare_dram_parameter("out", [CH, LBLK], F32, isOutput=True)

    with tile.TileContext(nc) as tc:
        with (
            tc.tile_pool(name="sb", bufs=1) as sb,
            tc.tile_pool(name="ps", bufs=1, space="PSUM") as ps,
        ):
            t_big = sb.tile([16, NCOL], F32)
            dma_in = nc.sync.dma_start(t_big[:], big[:], single_packet=True)
            fwT = t_big[:, C_FW:C_FW + 256]
            w2p = t_big[:, C_W2:C_W2 + NF]
            rhsw = t_big[0:2, C_RHSB:C_RHSB + 512].bitcast(BF16)  # [2,1024]
            lhs_up = t_big[0:2, C_LUPB:C_LUPB + 64].bitcast(BF16)  # [2,128]
            lhs_lo = t_big[0:2, C_LLOB:C_LLOB + 16].bitcast(BF16)  # [2,32]
            rv = t_big[:, C_RV:C_RV + 128]

            # ---- band masks (input-independent; hidden under DMA wait) ----
            ones = sb.tile([128, 128], BF16)
            nc.gpsimd.memset(ones[:], 1.0)
            btmp = sb.tile([128, 128], BF16)
            band_up = sb.tile([128, 128], BF16)
            band_lo = sb.tile([LO, 128], BF16)
            # up fold: partition=jl, free=p; keep iff 0 <= jl - p <= W-1
            nc.gpsimd.affine_select(btmp[:], ones[:], [[-1, 128]],
                                    Alu.is_ge, 0.0, base=0,
                                    channel_multiplier=1)
            nc.gpsimd.affine_select(band_up[:], btmp[:], [[1, 128]],
                                    Alu.is_ge, 0.0, base=W - 1,
                                    channel_multiplier=-1)
            # lo fold: jl = 128 + q; jl - p >= 0 is vacuous, keep iff
            # (W-1) - (128+q) + p >= 0
            last_gp = nc.gpsimd.affine_select(band_lo[:], ones[0:LO, :],
                                              [[1, 128]], Alu.is_ge, 0.0,
                                              base=W - 1 - 128,
                                              channel_multiplier=-1)

            # ---- G[jl, k*16+o], both folds, PSUM -> bf16 SBUF on scalar ----
            p_g = ps.tile([128, 2 * NF], F32)
            nc.tensor.matmul(p_g[:, 0:NF], fwT[:, 0:128], w2p[:],
                             start=True, stop=True)
            nc.tensor.matmul(p_g[0:LO, NF:2 * NF], fwT[:, 128:128 + LO],
                             w2p[:], start=True, stop=True)
            g_up = sb.tile([128, NF], BF16)
            g_lo = sb.tile([LO, NF], BF16)
            nc.scalar.copy(g_up[:], p_g[:, 0:NF])
            last_sc = nc.scalar.copy(g_lo[:], p_g[0:LO, NF:2 * NF])

            # ---- observers (single-wait discipline, see module docstring) --
            # PE absorbs the gpsimd band sem (covers both bands).
            p_obs = ps.tile([1, 8], F32)
            nc.tensor.matmul(p_obs[0:1, 0:1], band_up[0:1, 0:1],
                             band_up[0:1, 0:1], start=True, stop=True)
            # Vector absorbs the DMA sem then the gpsimd sem.
            obs_v = sb.tile([1, 2], F32)
            nc.vector.tensor_copy(obs_v[:, 0:1], t_big[0:1, 0:1])
            nc.vector.tensor_copy(obs_v[:, 1:2], band_up[0:1, 0:1])

            # ---- PA[jl, (k,p)] = w1k*dt + b1k, K=2 bf16 matmuls ----
            p_au = ps.tile([128, PAW], F32)
            p_al = ps.tile([LO, PAW], F32)
            nc.tensor.matmul(p_au[:, 0:512], lhs_up[:], rhsw[:, 0:512],
                             start=True, stop=True)
            nc.tensor.matmul(p_au[:, 512:1024], lhs_up[:], rhsw[:, 512:1024],
                             start=True, stop=True)
            nc.tensor.matmul(p_al[:, 0:512], lhs_lo[:, 0:LO], rhsw[:, 0:512],
                             start=True, stop=True)
            nc.tensor.matmul(p_al[:, 512:1024], lhs_lo[:, 0:LO],
                             rhsw[:, 512:1024], start=True, stop=True)

            # ---- A = max(PA,0) * band, fused, bf16 out, on vector ----
            a_up = sb.tile([128, PAW], BF16)
            a_lo = sb.tile([LO, PAW], BF16)
            bu3 = band_up[:].unsqueeze(1).broadcast_to((128, 4, 128))
            bl3 = band_lo[:].unsqueeze(1).broadcast_to((LO, 4, 128))
            stt = []
            for h in range(2):
                s = slice(h * 512, (h + 1) * 512)
                stt.append(nc.vector.scalar_tensor_tensor(
                    a_up[:, s].rearrange("a (k p) -> a k p", p=128),
                    p_au[:, s].rearrange("a (k p) -> a k p", p=128),
                    0.0, bu3, Alu.max, Alu.mult))
            for h in range(2):
                s = slice(h * 512, (h + 1) * 512)
                stt.append(nc.vector.scalar_tensor_tensor(
                    a_lo[:, s].rearrange("a (k p) -> a k p", p=128),
                    p_al[:, s].rearrange("a (k p) -> a k p", p=128),
                    0.0, bl3, Alu.max, Alu.mult))

            # ---- out^T[o,p] = sum_{k,jl} G[jl,ko] * A_k[jl,p] ----
            # G slices STATIONARY (LDWEIGHTS P=16), A moving (N=128 bf16).
            p_out = ps.tile([CH, 128], F32)
            nc.tensor.matmul(p_out[:], g_up[:, HID * CH:NF], band_up[:],
                             start=True, stop=False)
            nc.tensor.matmul(p_out[:], g_lo[:, HID * CH:NF], band_lo[:],
                             start=False, stop=False)
            for k in range(HID):          # up fold, gated on stt[0]/stt[1]
                nc.tensor.matmul(p_out[:], g_up[:, k * CH:(k + 1) * CH],
                                 a_up[:, k * 128:(k + 1) * 128],
                                 start=False, stop=False)
            last_pe = None
            for k in range(HID):          # lo fold, gated on stt[2]/stt[3]
                last_pe = nc.tensor.matmul(
                    p_out[:], g_lo[:, k * CH:(k + 1) * CH],
                    a_lo[:, k * 128:(k + 1) * 128],
                    start=False, stop=(k == HID - 1))

            # ---- row-validity fold + store (host transposes back) ----
            o_sb = sb.tile([CH, 128], F32)
            last_ve = nc.vector.tensor_mul(o_sb[:], p_out[:], rv)
            dma_out = nc.sync.dma_start(out_d[:], o_sb[:])

            # The kernel-tail drain waits on every sem; TRN2 instructions
            # encode one wait each, so pre-satisfy with single-wait SP nops.
            for prod in (dma_in, dma_out, last_gp, last_sc, last_ve,
                         last_pe):
                nop = nc.sync.nop(nofuse=True, hint="predrain_observer")
                add_dep_helper(nop.ins, prod.ins, sync=True,
                               reason="pre-drain single-wait observer")
    return nc


def kernel(times, features, lengths, true_ids, sim_size, w1, b1, w2, b2):
    global LAST
    times = np.ascontiguousarray(np.asarray(times, dtype=np.float32))
    features = np.ascontiguousarray(np.asarray(features, dtype=np.float32))
    lengths = np.asarray(lengths)
    true_ids = np.asarray(true_ids)
    sim = int(np.asarray(sim_size))
    w1 = np.asarray(w1, dtype=np.float32).reshape(-1)
    b1 = np.asarray(b1, dtype=np.float32).reshape(-1)
    w2 = np.asarray(w2, dtype=np.float32)
    b2 = np.asarray(b2, dtype=np.float32)

    W = (sim + 1) * KS
    WIN = LBLK + W - 1
    LO = WIN - 128

    if W not in _prog_cache:
        _prog_cache[W] = _build(W)
    nc = _prog_cache[W]

    # W2p[i, k*16+o] = w2[k, i*16+o]; cols 128:144 = b2 reshaped (16,16)
    w2p = np.concatenate(
        [w2.reshape(HID, CH, CH).transpose(1, 0, 2).reshape(CH, HID * CH),
         b2.reshape(CH, CH)], axis=1).astype(np.float32)

    in_maps = []
    for core in range(NCORES):
        b, blk = divmod(core, NBLK)
        l0 = blk * LBLK
        idx = np.arange(l0 - (W - 1), l0 + LBLK)
        valid = idx >= 0
        idxc = np.clip(idx, 0, L - 1)
        t_win = np.where(valid, times[b, idxc], 0.0).astype(np.float32)
        tiw = (true_ids[b, idxc] & valid).astype(np.float32)
        feat_w = (np.where(valid[:, None], features[b, idxc, :], 0.0)
                  * tiw[:, None]).astype(np.float32)
        t_row = times[b, l0:l0 + LBLK].astype(np.float32)
        rv = (np.arange(l0, l0 + LBLK) <=
              (sim + 1) * (int(lengths[b]) - 1)).astype(np.float32)

        big = np.zeros((16, NCOL), np.float32)
        big[:, C_FW:C_FW + 128] = feat_w[0:128].T
        big[:, C_FW + 128:C_FW + 128 + LO] = feat_w[128:WIN].T
        big[:, C_W2:C_W2 + NF] = w2p
        pa_rows = np.stack([
            (w1[:, None] * t_row[None, :] + b1[:, None]).reshape(-1),
            np.repeat(-w1, 128)])
        big[0:2, C_RHSB:C_RHSB + 512] = _pack_bf16(pa_rows)
        lu = np.stack([np.ones(128, np.float32), t_win[0:128]])
        big[0:2, C_LUPB:C_LUPB + 64] = _pack_bf16(lu)
        ll = np.zeros((2, 32), np.float32)
        ll[0, 0:LO] = 1.0
        ll[1, 0:LO] = t_win[128:WIN]
        big[0:2, C_LLOB:C_LLOB + 16] = _pack_bf16(ll)
        big[:, C_RV:C_RV + 128] = rv[None, :]
        in_maps.append({"big": big})

    res = run_bass_kernel_spmd(nc, in_maps, core_ids=list(range(NCORES)),
                               trace=TRACE)
    LAST = res

    out = np.zeros((BS, L, CH), np.float32)
    for core in range(NCORES):
        b, blk = divmod(core, NBLK)
        out[b, blk * LBLK:(blk + 1) * LBLK, :] = res.results[core]["out"].T
    return out


# revision 15
# speedup vs baseline: 1.6057x; 1.0067x over previous
"""Trainium2 Bass kernel for nn_ContConv1dDenseSim (banded continuous conv).

Math (reference):
  dt[b,l,j] = times[b,l]-times[b,j], masked to a causal band j in [l-W+1, l]
  (W = (sim_size+1)*kernel_size = 30), true_ids[b,j], and a row-validity mask.
  h = relu(dt*w1+b1)  (8 hidden), kv = (h@w2+b2) masked, reshaped (16,16)
  out[b,l,o] = sum_{j,i} features[b,j,i] * kv[b,l,j,i,o]

Factorization:
  G[j,k*16+o] = sum_i (f[j,i]*tiw[j]) * W2p[i,k*16+o]   (k=0..7; col 128+o is
                the b2 bias channel)
  A_k[jl,p]   = band(jl,p) * relu(w1k*dt[p,jl] + b1k)
  out[p,o]    = rv[p] * sum_{k,jl} A_k[jl,p] * G[jl,k*16+o]

Sharding: 8 cores = 2 batches x 4 query-row blocks of 128. Each core sees a
157-column key window (128 "up" + 29 "lo" fold) and emits out^T (16,128);
the host transposes back.

Device program (per core), all moving matmul operands in BF16 (1 cyc/col on
the PE vs 2 for fp32):
  - ONE packed input DMA (16 x 1120 f32; bf16 sections bit-packed on host and
    bitcast on device).
  - band masks from 3 gpsimd affine_selects (bf16 [128,128] up, [29,128] lo),
    generated during the input-DMA latency window.
  - G via 2 PE matmuls (featT*tiw stationary fp32, W2p moving), PSUM->SBUF
    bf16 copies on the scalar engine.
  - the whole pre-activation PA[jl,(k,p)] = w1k*dt+b1k via K=2 bf16 matmuls
    (lhsT = [ones; t_win], moving = host-packed [w1k*t_row+b1k ; -w1k] rows).
  - fused mask+relu on vector: A = max(PA,0)*band per 512-wide half, bf16
    out, with the [*,128] band broadcast across k-blocks by a stride-0 AP.
  - contraction with G STATIONARY (LDWEIGHTS P=16) and A MOVING (N=128 bf16):
    18 PSUM-accumulated matmuls into out^T[16,128].
  - rv row-validity fold via one vector multiply, DMA out.

NOTE: TRN2 engine instructions encode a single sync-wait, so the program is
ordered so each engine's first touch of any foreign-produced tensor has
exactly one new cross-engine dependency (tiny observer ops where needed), and
the Tile kernel-tail drain is pre-satisfied by single-wait SP nops.
"""

import numpy as np
import ml_dtypes
import concourse.bass as bass
import concourse.tile as tile
import concourse.mybir as mybir
from concourse.bass_utils import run_bass_kernel_spmd
from concourse.tile_rust import add_dep_helper

F32 = mybir.dt.float32
BF16 = mybir.dt.bfloat16
Alu = mybir.AluOpType

BS, L, CH, HID, KS = 2, 512, 16, 8, 5
LBLK = 128                      # query rows per core
NBLK = L // LBLK                # 4
NCORES = BS * NBLK              # 8
NF = (HID + 1) * CH             # 144 G columns (8 hidden + b2 channel)
PAW = HID * LBLK                # 1024 wide pre-activation columns

# packed input column layout (f32 columns; *B sections hold 2 bf16 per col)
C_FW = 0                        # [16, 256]  featT*tiw, folded (up|lo+pad)
C_W2 = 256                      # [16, 144]  W2p
C_RHSB = 400                    # [2, 512]   PA moving rows, bf16-packed
C_LUPB = 912                    # [2, 64]    PA lhsT up (ones; t_win), bf16
C_LLOB = 976                    # [2, 16]    PA lhsT lo, bf16-packed
C_RV = 992                      # [16, 128]  row-validity (replicated)
NCOL = 1120

TRACE = False
LAST = None

_prog_cache = {}


def _pack_bf16(x):
    """Pack a float array into f32 'containers', 2 bf16 per f32 column."""
    bf = np.ascontiguousarray(x.astype(ml_dtypes.bfloat16))
    v = bf.view(np.uint16)
    u32 = (v[..., 0::2].astype(np.uint32)
           | (v[..., 1::2].astype(np.uint32) << 16))
    return u32.view(np.float32)


def _build(W):
    WIN = LBLK + W - 1          # 157
    LO = WIN - 128              # 29
    nc = bass.Bass(trn_type="TRN2")

    big = nc.declare_dram_parameter("big", [16, NCOL], F32, isOutput=False)
    out_d = nc.declare_dram_parameter("out", [CH, LBLK], F32, isOutput=True)

    with tile.TileContext(nc) as tc:
        with (
            tc.tile_pool(name="sb", bufs=1) as sb,
            tc.tile_pool(name="ps", bufs=1, space="PSUM") as ps,
        ):
            t_big = sb.tile([16, NCOL], F32)
            dma_in = nc.sync.dma_start(t_big[:], big[:], single_packet=True)
            fwT = t_big[:, C_FW:C_FW + 256]
            w2p = t_big[:, C_W2:C_W2 + NF]
            rhsw = t_big[0:2, C_RHSB:C_RHSB + 512].bitcast(BF16)  # [2,1024]
            lhs_up = t_big[0:2, C_LUPB:C_LUPB + 64].bitcast(BF16)  # [2,128]
            lhs_lo = t_big[0:2, C_LLOB:C_LLOB + 16].bitcast(BF16)  # [2,32]
            rv = t_big[:, C_RV:C_RV + 128]

            # ---- band masks (input-independent; hidden under DMA wait) ----
            ones = sb.tile([128, 128], BF16)
            nc.gpsimd.memset(ones[:], 1.0)
            btmp = sb.tile([128, 128], BF16)
            band_up = sb.tile([128, 128], BF16)
            band_lo = sb.tile([LO, 128], BF16)
            # up fold: partition=jl, free=p; keep iff 0 <= jl - p <= W-1
            nc.gpsimd.affine_select(btmp[:], ones[:], [[-1, 128]],
                                    Alu.is_ge, 0.0, base=0,
                                    channel_multiplier=1)
            nc.gpsimd.affine_select(band_up[:], btmp[:], [[1, 128]],
                                    Alu.is_ge, 0.0, base=W - 1,
                                    channel_multiplier=-1)
            # lo fold: jl = 128 + q; jl - p >= 0 is vacuous, keep iff
            # (W-1) - (128+q) + p >= 0
            last_gp = nc.gpsimd.affine_select(band_lo[:], ones[0:LO, :],
                                              [[1, 128]], Alu.is_ge, 0.0,
                                              base=W - 1 - 128,
                                              channel_multiplier=-1)

            # ---- G[jl, k*16+o], both folds, PSUM -> bf16 SBUF on scalar ----
            p_g = ps.tile([128, 2 * NF], F32)
            nc.tensor.matmul(p_g[:, 0:NF], fwT[:, 0:128], w2p[:],
                             start=True, stop=True)
            nc.tensor.matmul(p_g[0:LO, NF:2 * NF], fwT[:, 128:128 + LO],
                             w2p[:], start=True, stop=True)
            g_up = sb.tile([128, NF], BF16)
            g_lo = sb.tile([LO, NF], BF16)
            nc.scalar.copy(g_up[:], p_g[:, 0:NF])
            last_sc = nc.scalar.copy(g_lo[:], p_g[0:LO, NF:2 * NF])

            # ---- observers (single-wait discipline, see module docstring) --
            # PE absorbs the gpsimd band sem (covers both bands).
            p_obs = ps.tile([1, 8], F32)
            nc.tensor.matmul(p_obs[0:1, 0:1], band_up[0:1, 0:1],
                             band_up[0:1, 0:1], start=True, stop=True)
            # Vector absorbs the DMA sem then the gpsimd sem.
            obs_v = sb.tile([1, 2], F32)
            nc.vector.tensor_copy(obs_v[:, 0:1], t_big[0:1, 0:1])
            nc.vector.tensor_copy(obs_v[:, 1:2], band_up[0:1, 0:1])

            # ---- PA[jl, (k,p)] = w1k*dt + b1k, K=2 bf16 matmuls ----
            p_au = ps.tile([128, PAW], F32)
            p_al = ps.tile([LO, PAW], F32)
            nc.tensor.matmul(p_au[:, 0:512], lhs_up[:], rhsw[:, 0:512],
                             start=True, stop=True)
            nc.tensor.matmul(p_au[:, 512:1024], lhs_up[:], rhsw[:, 512:1024],
                             start=True, stop=True)
            nc.tensor.matmul(p_al[:, 0:512], lhs_lo[:, 0:LO], rhsw[:, 0:512],
                             start=True, stop=True)
            nc.tensor.matmul(p_al[:, 512:1024], lhs_lo[:, 0:LO],
                             rhsw[:, 512:1024], start=True, stop=True)

            # ---- A = max(PA,0) * band, fused, bf16 out, on vector ----
            a_up = sb.tile([128, PAW], BF16)
            a_lo = sb.tile([LO, PAW], BF16)
            bu3 = band_up[:].unsqueeze(1).broadcast_to((128, 4, 128))
            bl3 = band_lo[:].unsqueeze(1).broadcast_to((LO, 4, 128))
            stt = []
            for h in range(2):
                s = slice(h * 512, (h + 1) * 512)
                stt.append(nc.vector.scalar_tensor_tensor(
                    a_up[:, s].rearrange("a (k p) -> a k p", p=128),
                    p_au[:, s].rearrange("a (k p) -> a k p", p=128),
                    0.0, bu3, Alu.max, Alu.mult))
            for h in range(2):
                s = slice(h * 512, (h + 1) * 512)
                stt.append(nc.vector.scalar_tensor_tensor(
                    a_lo[:, s].rearrange("a (k p) -> a k p", p=128),
                    p_al[:, s].rearrange("a (k p) -> a k p", p=128),
                    0.0, bl3, Alu.max, Alu.mult))

            # ---- out^T[o,p] = sum_{k,jl} G[jl,ko] * A_k[jl,p] ----
            # G slices STATIONARY (LDWEIGHTS P=16), A moving (N=128 bf16).
            p_out = ps.tile([CH, 128], F32)
            nc.tensor.matmul(p_out[:], g_up[:, HID * CH:NF], band_up[:],
                             start=True, stop=False)
            nc.tensor.matmul(p_out[:], g_lo[:, HID * CH:NF], band_lo[:],
                             start=False, stop=False)
            for k in range(HID):          # up fold, gated on stt[0]/stt[1]
                nc.tensor.matmul(p_out[:], g_up[:, k * CH:(k + 1) * CH],
                                 a_up[:, k * 128:(k + 1) * 128],
                                 start=False, stop=False)
            last_pe = None
            for k in range(HID):          # lo fold, gated on stt[2]/stt[3]
                last_pe = nc.tensor.matmul(
                    p_out[:], g_lo[:, k * CH:(k + 1) * CH],
                    a_lo[:, k * 128:(k + 1) * 128],
                    start=False, stop=(k == HID - 1))

            # ---- row-validity fold + store (host transposes back) ----
            o_sb = sb.tile([CH, 128], F32)
            last_ve = nc.vector.tensor_mul(o_sb[:], p_out[:], rv)
            dma_out = nc.sync.dma_start(out_d[:], o_sb[:],
                                        single_packet=True)

            # The kernel-tail drain waits on every sem; TRN2 instructions
            # encode one wait each, so pre-satisfy with single-wait SP nops.
            for prod in (dma_in, dma_out, last_gp, last_sc, last_ve,
                         last_pe):
                nop = nc.sync.nop(nofuse=True, hint="predrain_observer")
                add_dep_helper(nop.ins, prod.ins, sync=True,
                               reason="pre-drain single-wait observer")
    return nc


def kernel(times, features, lengths, true_ids, sim_size, w1, b1, w2, b2):
    global LAST
    times = np.ascontiguousarray(np.asarray(times, dtype=np.float32))
    features = np.ascontiguousarray(np.asarray(features, dtype=np.float32))
    lengths = np.asarray(lengths)
    true_ids = np.asarray(true_ids)
    sim = int(np.asarray(sim_size))
    w1 = np.asarray(w1, dtype=np.float32).reshape(-1)
    b1 = np.asarray(b1, dtype=np.float32).reshape(-1)
    w2 = np.asarray(w2, dtype=np.float32)
    b2 = np.asarray(b2, dtype=np.float32)

    W = (sim + 1) * KS
    WIN = LBLK + W - 1
    LO = WIN - 128

    if W not in _prog_cache:
        _prog_cache[W] = _build(W)
    nc = _prog_cache[W]

    # W2p[i, k*16+o] = w2[k, i*16+o]; cols 128:144 = b2 reshaped (16,16)
    w2p = np.concatenate(
        [w2.reshape(HID, CH, CH).transpose(1, 0, 2).reshape(CH, HID * CH),
         b2.reshape(CH, CH)], axis=1).astype(np.float32)

    in_maps = []
    for core in range(NCORES):
        b, blk = divmod(core, NBLK)
        l0 = blk * LBLK
        idx = np.arange(l0 - (W - 1), l0 + LBLK)
        valid = idx >= 0
        idxc = np.clip(idx, 0, L - 1)
        t_win = np.where(valid, times[b, idxc], 0.0).astype(np.float32)
        tiw = (true_ids[b, idxc] & valid).astype(np.float32)
        feat_w = (np.where(valid[:, None], features[b, idxc, :], 0.0)
                  * tiw[:, None]).astype(np.float32)
        t_row = times[b, l0:l0 + LBLK].astype(np.float32)
        rv = (np.arange(l0, l0 + LBLK) <=
              (sim + 1) * (int(lengths[b]) - 1)).astype(np.float32)

        big = np.zeros((16, NCOL), np.float32)
        big[:, C_FW:C_FW + 128] = feat_w[0:128].T
        big[:, C_FW + 128:C_FW + 128 + LO] = feat_w[128:WIN].T
        big[:, C_W2:C_W2 + NF] = w2p
        pa_rows = np.stack([
            (w1[:, None] * t_row[None, :] + b1[:, None]).reshape(-1),
            np.repeat(-w1, 128)])
        big[0:2, C_RHSB:C_RHSB + 512] = _pack_bf16(pa_rows)
        lu = np.stack([np.ones(128, np.float32), t_win[0:128]])
        big[0:2, C_LUPB:C_LUPB + 64] = _pack_bf16(lu)
        ll = np.zeros((2, 32), np.float32)
        ll[0, 0:LO] = 1.0
        ll[1, 0:LO] = t_win[128:WIN]
        big[0:2, C_LLOB:C_LLOB + 16] = _pack_bf16(ll)
        big[:, C_RV:C_RV + 128] = rv[None, :]
        in_maps.append({"big": big})

    res = run_bass_kernel_spmd(nc, in_maps, core_ids=list(range(NCORES)),
                               trace=TRACE)
    LAST = res

    out = np.zeros((BS, L, CH), np.float32)
    for core in range(NCORES):
        b, blk = divmod(core, NBLK)
        out[b, blk * LBLK:(blk + 1) * LBLK, :] = res.results[core]["out"].T
    return out


# revision 19
# speedup vs baseline: 1.6066x; 1.0005x over previous
"""Trainium2 Bass kernel for nn_ContConv1dDenseSim (banded continuous conv).

Math (reference):
  dt[b,l,j] = times[b,l]-times[b,j], masked to a causal band j in [l-W+1, l]
  (W = (sim_size+1)*kernel_size = 30), true_ids[b,j], and a row-validity mask.
  h = relu(dt*w1+b1)  (8 hidden), kv = (h@w2+b2) masked, reshaped (16,16)
  out[b,l,o] = sum_{j,i} features[b,j,i] * kv[b,l,j,i,o]

Factorization:
  G[j,k*16+o] = sum_i (f[j,i]*tiw[j]) * W2p[i,k*16+o]   (k=0..7; col 128+o is
                the b2 bias channel)
  A_k[jl,p]   = band(jl,p) * relu(w1k*dt[p,jl] + b1k)
  out[p,o]    = rv[p] * sum_{k,jl} A_k[jl,p] * G[jl,k*16+o]

Sharding: 8 cores = 2 batches x 4 query-row blocks of 128. Each core sees a
157-column key window (128 "up" + 29 "lo" fold) and emits out^T (16,128);
the host transposes back.

Device program (per core), all moving matmul operands in BF16 (1 cyc/col on
the PE vs 2 for fp32):
  - ONE packed input DMA (16 x 1120 f32; bf16 sections bit-packed on host and
    bitcast on device).
  - band masks from 3 gpsimd affine_selects (bf16 [128,128] up, [29,128] lo),
    generated during the input-DMA latency window.
  - G via 2 PE matmuls (featT*tiw stationary fp32, W2p moving), PSUM->SBUF
    bf16 copies on the scalar engine.
  - the whole pre-activation PA[jl,(k,p)] = w1k*dt+b1k via K=2 bf16 matmuls
    (lhsT = [ones; t_win], moving = host-packed [w1k*t_row+b1k ; -w1k] rows).
  - fused mask+relu on vector: A = max(PA,0)*band per 512-wide half, bf16
    out, with the [*,128] band broadcast across k-blocks by a stride-0 AP.
  - contraction with G STATIONARY (LDWEIGHTS P=16) and A MOVING (N=128 bf16):
    18 PSUM-accumulated matmuls into out^T[16,128].
  - rv row-validity fold via one vector multiply, DMA out.

NOTE: TRN2 engine instructions encode a single sync-wait, so the program is
ordered so each engine's first touch of any foreign-produced tensor has
exactly one new cross-engine dependency (tiny observer ops where needed), and
the Tile kernel-tail drain is pre-satisfied by single-wait SP nops.
"""

import numpy as np
import ml_dtypes
import concourse.bass as bass
import concourse.tile as tile
import concourse.mybir as mybir
from concourse.bass_utils import run_bass_kernel_spmd
from concourse.tile_rust import add_dep_helper

F32 = mybir.dt.float32
BF16 = mybir.dt.bfloat16
Alu = mybir.AluOpType

BS, L, CH, HID, KS = 2, 512, 16, 8, 5
LBLK = 128                      # query rows per core
NBLK = L // LBLK                # 4
NCORES = BS * NBLK              # 8
NF = (HID + 1) * CH             # 144 G columns (8 hidden + b2 channel)
PAW = HID * LBLK                # 1024 wide pre-activation columns

# packed input column layout (f32 columns; *B sections hold 2 bf16 per col).
# Cols [0:592) are the PA operands (rows 0:2 only) — DMA'd separately so the
# PE can start before the bulk section lands.
C_RHSB = 0                      # [2, 512]   PA moving rows, bf16-packed
C_LUPB = 512                    # [2, 64]    PA lhsT up (ones; t_win), bf16
C_LLOB = 576                    # [2, 16]    PA lhsT lo, bf16-packed
C_FW = 592                      # [16, 256]  featT*tiw, folded (up|lo+pad)
C_W2 = 848                      # [16, 144]  W2p
C_RV = 992                      # [16, 128]  row-validity (replicated)
C_SPLIT = 592
NCOL = 1120

TRACE = False
LAST = None

_prog_cache = {}


def _pack_bf16(x):
    """Pack a float array into f32 'containers', 2 bf16 per f32 column."""
    bf = np.ascontiguousarray(x.astype(ml_dtypes.bfloat16))
    v = bf.view(np.uint16)
    u32 = (v[..., 0::2].astype(np.uint32)
           | (v[..., 1::2].astype(np.uint32) << 16))
    return u32.view(np.float32)


def _build(W):
    WIN = LBLK + W - 1          # 157
    LO = WIN - 128              # 29
    nc = bass.Bass(trn_type="TRN2")

    big = nc.declare_dram_parameter("big", [16, NCOL], F32, isOutput=False)
    out_d = nc.declare_dram_parameter("out", [CH, LBLK], F32, isOutput=True)

    with tile.TileContext(nc) as tc:
        with (
            tc.tile_pool(name="sb", bufs=1) as sb,
            tc.tile_pool(name="ps", bufs=1, space="PSUM") as ps,
        ):
            t_big = sb.tile([16, NCOL], F32)
            dma_a = nc.sync.dma_start(t_big[0:2, 0:C_SPLIT],
                                      big[0:2, 0:C_SPLIT],
                                      single_packet=True)
            dma_b = nc.scalar.dma_start(t_big[:, C_SPLIT:NCOL],
                                        big[:, C_SPLIT:NCOL],
                                        single_packet=True)
            fwT = t_big[:, C_FW:C_FW + 256]
            w2p = t_big[:, C_W2:C_W2 + NF]
            rhsw = t_big[0:2, C_RHSB:C_RHSB + 512].bitcast(BF16)  # [2,1024]
            lhs_up = t_big[0:2, C_LUPB:C_LUPB + 64].bitcast(BF16)  # [2,128]
            lhs_lo = t_big[0:2, C_LLOB:C_LLOB + 16].bitcast(BF16)  # [2,32]
            rv = t_big[:, C_RV:C_RV + 128]

            # ---- band masks (input-independent; hidden under DMA wait) ----
            ones = sb.tile([128, 128], BF16)
            nc.gpsimd.memset(ones[:], 1.0)
            btmp = sb.tile([128, 128], BF16)
            band_up = sb.tile([128, 128], BF16)
            band_lo = sb.tile([LO, 128], BF16)
            # up fold: partition=jl, free=p; keep iff 0 <= jl - p <= W-1
            nc.gpsimd.affine_select(btmp[:], ones[:], [[-1, 128]],
                                    Alu.is_ge, 0.0, base=0,
                                    channel_multiplier=1)
            nc.gpsimd.affine_select(band_up[:], btmp[:], [[1, 128]],
                                    Alu.is_ge, 0.0, base=W - 1,
                                    channel_multiplier=-1)
            # lo fold: jl = 128 + q; jl - p >= 0 is vacuous, keep iff
            # (W-1) - (128+q) + p >= 0
            last_gp = nc.gpsimd.affine_select(band_lo[:], ones[0:LO, :],
                                              [[1, 128]], Alu.is_ge, 0.0,
                                              base=W - 1 - 128,
                                              channel_multiplier=-1)

            # ---- PA[jl, (k,p)] = w1k*dt + b1k, K=2 bf16 matmuls (needs
            # only the small dma_a section, so the PE starts early) ----
            p_au = ps.tile([128, PAW], F32)
            p_al = ps.tile([LO, PAW], F32)
            nc.tensor.matmul(p_au[:, 0:512], lhs_up[:], rhsw[:, 0:512],
                             start=True, stop=True)
            nc.tensor.matmul(p_au[:, 512:1024], lhs_up[:], rhsw[:, 512:1024],
                             start=True, stop=True)
            nc.tensor.matmul(p_al[:, 0:512], lhs_lo[:, 0:LO], rhsw[:, 0:512],
                             start=True, stop=True)
            nc.tensor.matmul(p_al[:, 512:1024], lhs_lo[:, 0:LO],
                             rhsw[:, 512:1024], start=True, stop=True)

            # ---- G[jl, k*16+o], both folds, PSUM -> bf16 SBUF on scalar ----
            p_g = ps.tile([128, 2 * NF], F32)
            nc.tensor.matmul(p_g[:, 0:NF], fwT[:, 0:128], w2p[:],
                             start=True, stop=True)
            nc.tensor.matmul(p_g[0:LO, NF:2 * NF], fwT[:, 128:128 + LO],
                             w2p[:], start=True, stop=True)
            g_up = sb.tile([128, NF], BF16)
            g_lo = sb.tile([LO, NF], BF16)
            nc.scalar.copy(g_up[:], p_g[:, 0:NF])
            last_sc = nc.scalar.copy(g_lo[:], p_g[0:LO, NF:2 * NF])

            # ---- observers (single-wait discipline, see module docstring) --
            # PE absorbs the gpsimd band sem (covers both bands).
            p_obs = ps.tile([1, 8], F32)
            nc.tensor.matmul(p_obs[0:1, 0:1], band_up[0:1, 0:1],
                             band_up[0:1, 0:1], start=True, stop=True)
            # Vector absorbs the dma_b sem then the gpsimd sem.
            obs_v = sb.tile([1, 2], F32)
            nc.vector.tensor_copy(obs_v[:, 0:1], t_big[0:1, C_FW:C_FW + 1])
            nc.vector.tensor_copy(obs_v[:, 1:2], band_up[0:1, 0:1])

            # ---- A = max(PA,0) * band, fused, bf16 out, on vector ----
            a_up = sb.tile([128, PAW], BF16)
            a_lo = sb.tile([LO, PAW], BF16)
            bu3 = band_up[:].unsqueeze(1).broadcast_to((128, 4, 128))
            bl3 = band_lo[:].unsqueeze(1).broadcast_to((LO, 4, 128))
            stt = []
            for h in range(2):
                s = slice(h * 512, (h + 1) * 512)
                stt.append(nc.vector.scalar_tensor_tensor(
                    a_up[:, s].rearrange("a (k p) -> a k p", p=128),
                    p_au[:, s].rearrange("a (k p) -> a k p", p=128),
                    0.0, bu3, Alu.max, Alu.mult))
            for h in range(2):
                s = slice(h * 512, (h + 1) * 512)
                stt.append(nc.vector.scalar_tensor_tensor(
                    a_lo[:, s].rearrange("a (k p) -> a k p", p=128),
                    p_al[:, s].rearrange("a (k p) -> a k p", p=128),
                    0.0, bl3, Alu.max, Alu.mult))

            # ---- out^T[o,p] = sum_{k,jl} G[jl,ko] * A_k[jl,p] ----
            # G slices STATIONARY (LDWEIGHTS P=16), A moving (N=128 bf16).
            p_out = ps.tile([CH, 128], F32)
            nc.tensor.matmul(p_out[:], g_up[:, HID * CH:NF], band_up[:],
                             start=True, stop=False)
            nc.tensor.matmul(p_out[:], g_lo[:, HID * CH:NF], band_lo[:],
                             start=False, stop=False)
            for k in range(HID):          # up fold, gated on stt[0]/stt[1]
                nc.tensor.matmul(p_out[:], g_up[:, k * CH:(k + 1) * CH],
                                 a_up[:, k * 128:(k + 1) * 128],
                                 start=False, stop=False)
            last_pe = None
            for k in range(HID):          # lo fold, gated on stt[2]/stt[3]
                last_pe = nc.tensor.matmul(
                    p_out[:], g_lo[:, k * CH:(k + 1) * CH],
                    a_lo[:, k * 128:(k + 1) * 128],
                    start=False, stop=(k == HID - 1))

            # ---- row-validity fold + store (host transposes back) ----
            o_sb = sb.tile([CH, 128], F32)
            last_ve = nc.vector.tensor_mul(o_sb[:], p_out[:], rv)
            dma_out = nc.sync.dma_start(out_d[:], o_sb[:],
                                        single_packet=True)

            # The kernel-tail drain waits on every sem; TRN2 instructions
            # encode one wait each, so pre-satisfy with single-wait SP nops.
            for prod in (dma_a, dma_b, dma_out, last_gp, last_sc, last_ve,
                         last_pe):
                nop = nc.sync.nop(nofuse=True, hint="predrain_observer")
                add_dep_helper(nop.ins, prod.ins, sync=True,
                               reason="pre-drain single-wait observer")
    return nc


def kernel(times, features, lengths, true_ids, sim_size, w1, b1, w2, b2):
    global LAST
    times = np.ascontiguousarray(np.asarray(times, dtype=np.float32))
    features = np.ascontiguousarray(np.asarray(features, dtype=np.float32))
    lengths = np.asarray(lengths)
    true_ids = np.asarray(true_ids)
    sim = int(np.asarray(sim_size))
    w1 = np.asarray(w1, dtype=np.float32).reshape(-1)
    b1 = np.asarray(b1, dtype=np.float32).reshape(-1)
    w2 = np.asarray(w2, dtype=np.float32)
    b2 = np.asarray(b2, dtype=np.float32)

    W = (sim + 1) * KS
    WIN = LBLK + W - 1
    LO = WIN - 128

    if W not in _prog_cache:
        _prog_cache[W] = _build(W)
    nc = _prog_cache[W]

    # W2p[i, k*16+o] = w2[k, i*16+o]; cols 128:144 = b2 reshaped (16,16)
    w2p = np.concatenate(
        [w2.reshape(HID, CH, CH).transpose(1, 0, 2).reshape(CH, HID * CH),
         b2.reshape(CH, CH)], axis=1).astype(np.float32)

    in_maps = []
    for core in range(NCORES):
        b, blk = divmod(core, NBLK)
        l0 = blk * LBLK
        idx = np.arange(l0 - (W - 1), l0 + LBLK)
        valid = idx >= 0
        idxc = np.clip(idx, 0, L - 1)
        t_win = np.where(valid, times[b, idxc], 0.0).astype(np.float32)
        tiw = (true_ids[b, idxc] & valid).astype(np.float32)
        feat_w = (np.where(valid[:, None], features[b, idxc, :], 0.0)
                  * tiw[:, None]).astype(np.float32)
        t_row = times[b, l0:l0 + LBLK].astype(np.float32)
        rv = (np.arange(l0, l0 + LBLK) <=
              (sim + 1) * (int(lengths[b]) - 1)).astype(np.float32)

        big = np.zeros((16, NCOL), np.float32)
        big[:, C_FW:C_FW + 128] = feat_w[0:128].T
        big[:, C_FW + 128:C_FW + 128 + LO] = feat_w[128:WIN].T
        big[:, C_W2:C_W2 + NF] = w2p
        pa_rows = np.stack([
            (w1[:, None] * t_row[None, :] + b1[:, None]).reshape(-1),
            np.repeat(-w1, 128)])
        big[0:2, C_RHSB:C_RHSB + 512] = _pack_bf16(pa_rows)
        lu = np.stack([np.ones(128, np.float32), t_win[0:128]])
        big[0:2, C_LUPB:C_LUPB + 64] = _pack_bf16(lu)
        ll = np.zeros((2, 32), np.float32)
        ll[0, 0:LO] = 1.0
        ll[1, 0:LO] = t_win[128:WIN]
        big[0:2, C_LLOB:C_LLOB + 16] = _pack_bf16(ll)
        big[:, C_RV:C_RV + 128] = rv[None, :]
        in_maps.append({"big": big})

    res = run_bass_kernel_spmd(nc, in_maps, core_ids=list(range(NCORES)),
                               trace=TRACE)
    LAST = res

    out = np.zeros((BS, L, CH), np.float32)
    for core in range(NCORES):
        b, blk = divmod(core, NBLK)
        out[b, blk * LBLK:(blk + 1) * LBLK, :] = res.results[core]["out"].T
    return out
